# revision 13
# baseline (speedup 1.0000x reference)
"""DLRM pairwise-interaction layer on 8 Trainium2 NeuronCores.

Computes, for each batch row b, the strict upper triangle of the Gram matrix
G_b = E_b @ E_b.T where E_b is (27 features, 128 dims), i.e. the reference

    interactions = einsum("bfd,bgd->bfg", E, E);  out = interactions[:, triu_i, triu_j]

Strategy (pure batch data-parallel, 2048 rows/core), bf16, UNPADDED stream:
  * Host transposes to (128, rows*27) bf16 -- no feature padding.  Total
    input DMA is 14.16 MB/core (vs 16.8 MB padded), the kernel's pacer.
  * 4 batch rows per matmul group: stationary = a contiguous 128-col window
    at stride 108 (the group's 108 cols + 20-col overlap into the next
    group; NumWeights==128 keeps fast-weight-load); moving = the group's
    108 cols (N=108).  The four wanted 27x27 diagonal blocks land at PSUM
    (parts 27q, cols 27q); junk partitions 108..127 are never read.
  * Engine PSUM reads must start at a 32-aligned partition; spans starting
    at partition 0 may be any size, and engine cost depends only on the
    free-dim size (partitions are parallel lanes).  So every extraction
    copy starts at partition 0 with span {27,54,81,108} per q.
  * Per 64-row pass-tile (16 matmuls, 4 PSUM banks): VectorE extracts banks
    0-1 and ScalarE banks 2-3 concurrently (different banks -> legal), 4
    q-copies each, col slices 27q+1..27q+26 (g=0 dropped; only f<g needed).
    psum bufs=2 lets PE fill banks 4-7 meanwhile.
  * Out tile is laid out q-outermost so the per-q output DMA (slicing the
    27 good partitions) is one contiguous multi-KB run per partition.
    Output is 2.8 MB/core.
  * Chunk sizes ramp 64..256..64 so the first matmul starts after ~0.4 MB
    of DMA instead of 2 MB, and the tail drains quickly.
"""

import numpy as np

B = 16384
F = 27
GO = 26                      # g columns kept per block (g = 1..26)
D = 128
NCORES = 8
BLOC = B // NCORES           # 2048 batch rows per core
CHUNK_ROWS = [64, 64, 128] + [256] * 6 + [128, 64, 64]   # sums to 2048
assert sum(CHUNK_ROWS) == BLOC
NTILE = BLOC // 64           # 32 pass-tiles of 64 rows
ET_COLS = BLOC * F + 128     # unpadded stream + tail pad for last window

QSPAN = [27, 54, 81, 108]    # psum partition span per q (start always 0)

_TRIU_I, _TRIU_J = np.triu_indices(F, k=1)

_compiled = None


def _build():
    import concourse.bacc as bacc
    import concourse.mybir as mybir
    from concourse.tile import TileContext

    f32 = mybir.dt.float32
    bf16 = mybir.dt.bfloat16
    nc = bacc.Bacc(None, target_bir_lowering=False)

    et = nc.dram_tensor("et", [D, ET_COLS], bf16, kind="ExternalInput")
    y = nc.dram_tensor("y", [4, F, NTILE, 4, 4, GO], bf16,
                       kind="ExternalOutput")

    from concourse.ap import AP

    with TileContext(nc) as tc:
        with (
            tc.tile_pool(name="inp", bufs=8) as inp,
            tc.tile_pool(name="outp", bufs=2) as outp,
            tc.tile_pool(name="packp", bufs=2) as packp,
            tc.tile_pool(name="psum", bufs=1, space="PSUM") as psum,
        ):
            # one persistent 8-bank PSUM tile; banks indexed explicitly so
            # dependency tracking is per-bank (slice-level), letting the PE
            # refill a bank as soon as its extraction copy completes.
            ps = psum.tile([D, 8, 4, 128], f32)
            PP = 8 * 4 * 128            # psum per-partition pitch (elems)
            OP = 4 * 8 * 4 * 4 * GO     # out_t per-partition pitch (elems)
            TCOLS = 64 * F              # 1728 cols per pass-tile
            CSTART = [sum(CHUNK_ROWS[:i]) for i in range(len(CHUNK_ROWS))]
            GSIZES = [8, 8, 8, 8]            # output tile-groups (sum 32)
            GSTART = set()
            GEND = set()
            acc = 0
            for gs in GSIZES:
                GSTART.add(acc)
                acc += gs
                GEND.add(acc)

            in_tiles = {}

            def emit_in(ci):
                rows_c = CHUNK_ROWS[ci]
                in_t = inp.tile([D, 256 * F + 20], bf16)
                c0 = CSTART[ci] * F
                nc.sync.dma_start(
                    in_t[:, :rows_c * F + 20],
                    et[:, c0:c0 + rows_c * F + 20],
                )
                in_tiles[ci] = in_t

            # prefetch 4 chunks; inside iteration c, chunk c+4's input DMA is
            # emitted BEFORE chunk c's output DMAs so the Sync ring's FIFO
            # never stalls input prefetch behind an output's sem-wait.
            for ci in range(4):
                emit_in(ci)
            for c, rows_c in enumerate(CHUNK_ROWS):
                if c + 4 < len(CHUNK_ROWS):
                    emit_in(c + 4)
                npass = rows_c // 64
                in_t = in_tiles.pop(c)
                for hh in range(npass):
                    tctr = CSTART[c] // 64 + hh
                    if tctr in GSTART:
                        # part, q, tile-in-group(<=8), bank, slot, g
                        out_t = outp.tile([D, 4, 8, 4, 4, GO], bf16)
                        g0 = tctr
                    b0 = 4 * (tctr % 2)          # bank group for this tile
                    for m in range(16):
                        g = 16 * hh + m
                        stat = in_t[:, 108 * g:108 * g + 128]
                        mov = in_t[:, 108 * g:108 * g + 108]
                        nc.tensor.matmul(ps[:, b0 + m // 4, m % 4, 0:108],
                                         stat, mov, start=True, stop=True)
                    # one merged copy per engine per tile: dims
                    # [part, bankslot(8), q(4), g(26)] -- bank stride 512 =
                    # 4*slot stride and out bk stride 104 = 4*s stride, so
                    # (bank, slot) merge into one AP dim.  V reads banks
                    # b0..b0+1 while S reads b0+2..b0+3 (different banks),
                    # and each half-tile frees for the PE independently.
                    for half in range(2):
                        sb = ps[0:108, b0 + 2 * half, 0, :]
                        csrc = AP(tensor=sb.tensor, offset=sb.offset + 1,
                                  ap=[[PP, 108], [128, 8], [27, 4], [1, GO]])
                        db = out_t[0:108, 0, tctr - g0, 2 * half, 0, :]
                        cdst = AP(tensor=db.tensor, offset=db.offset,
                                  ap=[[OP, 108], [GO, 8],
                                      [8 * 4 * 4 * GO, 4], [1, GO]])
                        if half == 0:
                            nc.vector.tensor_copy(cdst, csrc)
                        else:
                            nc.scalar.copy(cdst, csrc)
                    # group done: compact each q-slot in SBUF (bf16 4x
                    # copy, same partitions) so the output DMA spans 108
                    # partitions and spreads evenly over all 16 DMA queues
                    if tctr + 1 in GEND:
                        gn = tctr + 1 - g0
                        pk = packp.tile([D, 8, 4, 4, GO], bf16)
                        # engine lane ranges must start at partition 0, so
                        # copy q in DESCENDING order with span 27q+27: later
                        # (smaller) copies overwrite the junk prefix, leaving
                        # partition p holding its own q = p//27 slot.
                        for q in (3, 2, 1, 0):
                            span = 27 * q + F
                            cpy = (nc.vector.tensor_copy if q % 2 == 0
                                   else nc.scalar.copy)
                            cpy(pk[0:span, :gn, :, :, :],
                                out_t[0:span, q, :gn, :, :, :])
                        pb = pk[0:108, 0, 0, 0, :]
                        psrc = AP(tensor=pb.tensor, offset=pb.offset,
                                  ap=[[8 * 4 * 4 * GO, 108],
                                      [1, gn * 4 * 4 * GO]])
                        yb = y[0, :, g0:g0 + gn, :, :, :]
                        ydst = AP(tensor=yb.tensor, offset=yb.offset,
                                  ap=[[NTILE * 4 * 4 * GO, 108],
                                      [1, gn * 4 * 4 * GO]])
                        nc.sync.dma_start(ydst, psrc)

    nc.compile()
    return nc


def _get_compiled():
    global _compiled
    if _compiled is None:
        _compiled = _build()
    return _compiled


def _prep_inputs(embeddings: np.ndarray):
    """Full (B, 27, 128) fp32 -> per-core unpadded bf16 (128, ET_COLS)."""
    import ml_dtypes

    bf16 = ml_dtypes.bfloat16
    e = np.asarray(embeddings, dtype=np.float32)
    # (D, B, F) bf16
    eT = np.ascontiguousarray(e.transpose(2, 0, 1)).astype(bf16)
    in_maps = []
    for c in range(NCORES):
        etc = np.zeros((D, ET_COLS), dtype=bf16)
        etc[:, :BLOC * F] = eT[:, c * BLOC:(c + 1) * BLOC, :].reshape(
            D, BLOC * F
        )
        in_maps.append({"et": etc})
    return in_maps


def _decode_core(yv: np.ndarray) -> np.ndarray:
    """(4, 27, NTILE, 4, 4, GO) bf16 -> (BLOC, 351) fp32."""
    g = np.asarray(yv).astype(np.float32)
    # row = 64*t + 16*bk + 4*s + q ; g[q, f, t, bk, s, j] = G[row, f, j+1]
    g = g.transpose(2, 3, 4, 0, 1, 5).reshape(BLOC, F, GO)
    return g[:, _TRIU_I, _TRIU_J - 1]


def kernel(embeddings: np.ndarray) -> np.ndarray:
    from concourse.bass_utils import run_bass_kernel_spmd

    nc = _get_compiled()
    in_maps = _prep_inputs(embeddings)
    res = run_bass_kernel_spmd(nc, in_maps, core_ids=list(range(NCORES)))

    out = np.empty((B, len(_TRIU_I)), dtype=np.float32)
    for c in range(NCORES):
        out[c * BLOC:(c + 1) * BLOC] = _decode_core(res.results[c]["y"])
    return out


# revision 14
# speedup vs baseline: 1.2081x; 1.2081x over previous
"""DLRM pairwise-interaction layer on 8 Trainium2 NeuronCores.

Computes, for each batch row b, the strict upper triangle of the Gram matrix
G_b = E_b @ E_b.T where E_b is (27 features, 128 dims), i.e. the reference

    interactions = einsum("bfd,bgd->bfg", E, E);  out = interactions[:, triu_i, triu_j]

Strategy (pure batch data-parallel, 2048 rows/core), bf16, UNPADDED stream:
  * Host transposes to (128, rows*27) bf16 -- no feature padding.  Total
    input DMA is 14.16 MB/core (vs 16.8 MB padded); in+out DMA share the
    16 queues (queues within an engine never overlap), so total DMA bytes
    set the kernel's pace.
  * 4 batch rows per matmul group: stationary = a contiguous 128-col window
    at stride 108 (the group's 108 cols + 20-col overlap into the next
    group; NumWeights==128 keeps fast-weight-load); moving = the group's
    108 cols (N=108).  The four wanted 27x27 diagonal blocks land at PSUM
    (parts 27q, cols 27q); junk partitions 108..127 are never read.
  * Engine (PSUM or SBUF) lane ranges must start at a 32-aligned partition;
    ranges starting at partition 0 may have any span, and engine cost
    depends only on free-dim size (partitions are parallel lanes).  So all
    extraction copies start at partition 0 with span up to 108.
  * PSUM is one persistent 8-bank tile with banks indexed explicitly, so
    dependency tracking is slice-level and the PE can refill a bank group
    as soon as its extraction copy completes.  Pass-tiles (64 rows, 16
    matmuls) alternate bank groups 0-3 / 4-7.
  * Extraction is one merged copy per engine per pass-tile with AP dims
    [part(108), bankslot(8), q(4, col stride 27), g(26)] -- bank stride
    512 = 4*slot stride so (bank,slot) merge; the q dim maps to dst stride
    3328 and junk lanes land in junk dst areas.  VectorE reads banks
    b0..b0+1 while ScalarE reads b0+2..b0+3 (different banks, legal in
    parallel), g = 1..26 only (f<g pairs).
  * Output accumulates in SBUF per 8-tile group (512 rows); per-q output
    DMAs (clean 27-partition slices, contiguous 6.5 KB runs) go on the
    Sync ring AFTER the next chunks' input DMAs so the ring's FIFO never
    stalls input prefetch behind an output's semaphore wait.  Output is
    2.8 MB/core.
  * Input chunk sizes ramp 64..256..64 so the first matmul starts early
    and the tail drains quickly; 4-chunk input prefetch (bufs=4).
"""

import numpy as np

B = 16384
F = 27
GO = 26                      # g columns kept per block (g = 1..26)
D = 128
NCORES = 8
BLOC = B // NCORES           # 2048 batch rows per core
CHUNK_ROWS = [64, 64, 128] + [256] * 6 + [128, 64, 64]   # sums to 2048
assert sum(CHUNK_ROWS) == BLOC
NTILE = BLOC // 64           # 32 pass-tiles of 64 rows
ET_COLS = BLOC * F + 128     # unpadded stream + tail pad for last window

_TRIU_I, _TRIU_J = np.triu_indices(F, k=1)

_compiled = None


def _build():
    import concourse.bacc as bacc
    import concourse.mybir as mybir
    from concourse.tile import TileContext
    from concourse.ap import AP

    f32 = mybir.dt.float32
    bf16 = mybir.dt.bfloat16
    nc = bacc.Bacc(None, target_bir_lowering=False)

    et = nc.dram_tensor("et", [D, ET_COLS], bf16, kind="ExternalInput")
    y = nc.dram_tensor("y", [4, F, NTILE, 4, 4, GO], bf16,
                       kind="ExternalOutput")

    with TileContext(nc) as tc:
        with (
            tc.tile_pool(name="inp", bufs=4) as inp,
            tc.tile_pool(name="outp", bufs=2) as outp,
            tc.tile_pool(name="psum", bufs=1, space="PSUM") as psum,
        ):
            ps = psum.tile([D, 8, 4, 128], f32)
            PP = 8 * 4 * 128            # psum per-partition pitch (elems)
            OP = 4 * 8 * 4 * 4 * GO     # out_t per-partition pitch (elems)
            CSTART = [sum(CHUNK_ROWS[:i]) for i in range(len(CHUNK_ROWS))]

            in_tiles = {}

            def emit_in(ci):
                rows_c = CHUNK_ROWS[ci]
                in_t = inp.tile([D, 256 * F + 20], bf16)
                c0 = CSTART[ci] * F
                nc.sync.dma_start(
                    in_t[:, :rows_c * F + 20],
                    et[:, c0:c0 + rows_c * F + 20],
                )
                in_tiles[ci] = in_t

            # prefetch 4 chunks; inside iteration c, chunk c+4's input DMA is
            # emitted BEFORE chunk c's output DMAs so the Sync ring's FIFO
            # never stalls input prefetch behind an output's sem-wait.
            for ci in range(4):
                emit_in(ci)
            out_t = None
            for c, rows_c in enumerate(CHUNK_ROWS):
                if c + 4 < len(CHUNK_ROWS):
                    emit_in(c + 4)
                npass = rows_c // 64
                in_t = in_tiles.pop(c)
                for hh in range(npass):
                    tctr = CSTART[c] // 64 + hh
                    if tctr % 8 == 0:
                        # part, q, tile-in-group(8), bank, slot, g
                        out_t = outp.tile([D, 4, 8, 4, 4, GO], bf16)
                    b0 = 4 * (tctr % 2)          # bank group for this tile
                    for m in range(16):
                        g = 16 * hh + m
                        stat = in_t[:, 108 * g:108 * g + 128]
                        mov = in_t[:, 108 * g:108 * g + 108]
                        nc.tensor.matmul(ps[:, b0 + m // 4, m % 4, 0:108],
                                         stat, mov, start=True, stop=True)
                    # one merged copy per engine per tile
                    for half in range(2):
                        sb = ps[0:108, b0 + 2 * half, 0, :]
                        csrc = AP(tensor=sb.tensor, offset=sb.offset + 1,
                                  ap=[[PP, 108], [128, 8], [27, 4], [1, GO]])
                        db = out_t[0:108, 0, tctr % 8, 2 * half, 0, :]
                        cdst = AP(tensor=db.tensor, offset=db.offset,
                                  ap=[[OP, 108], [GO, 8],
                                      [8 * 4 * 4 * GO, 4], [1, GO]])
                        if half == 0:
                            nc.vector.tensor_copy(cdst, csrc)
                        else:
                            nc.scalar.copy(cdst, csrc)
                    # per-q output DMAs for each 8-tile group (512 rows)
                    if tctr % 8 == 7:
                        gi = tctr // 8
                        for q in range(4):
                            nc.sync.dma_start(
                                y[q, :, 8 * gi:8 * gi + 8, :, :, :],
                                out_t[27 * q:27 * q + F, q, :, :, :, :],
                            )

    nc.compile()
    return nc


def _get_compiled():
    global _compiled
    if _compiled is None:
        _compiled = _build()
    return _compiled


def _prep_inputs(embeddings: np.ndarray):
    """Full (B, 27, 128) fp32 -> per-core unpadded bf16 (128, ET_COLS)."""
    import ml_dtypes

    bf16 = ml_dtypes.bfloat16
    e = np.asarray(embeddings, dtype=np.float32)
    # (D, B, F) bf16
    eT = np.ascontiguousarray(e.transpose(2, 0, 1)).astype(bf16)
    in_maps = []
    for c in range(NCORES):
        etc = np.zeros((D, ET_COLS), dtype=bf16)
        etc[:, :BLOC * F] = eT[:, c * BLOC:(c + 1) * BLOC, :].reshape(
            D, BLOC * F
        )
        in_maps.append({"et": etc})
    return in_maps


def _decode_core(yv: np.ndarray) -> np.ndarray:
    """(4, 27, NTILE, 4, 4, GO) bf16 -> (BLOC, 351) fp32."""
    g = np.asarray(yv).astype(np.float32)
    # row = 64*t + 16*bk + 4*s + q ; g[q, f, t, bk, s, j] = G[row, f, j+1]
    g = g.transpose(2, 3, 4, 0, 1, 5).reshape(BLOC, F, GO)
    return g[:, _TRIU_I, _TRIU_J - 1]


def kernel(embeddings: np.ndarray) -> np.ndarray:
    from concourse.bass_utils import run_bass_kernel_spmd

    nc = _get_compiled()
    in_maps = _prep_inputs(embeddings)
    res = run_bass_kernel_spmd(nc, in_maps, core_ids=list(range(NCORES)))

    out = np.empty((B, len(_TRIU_I)), dtype=np.float32)
    for c in range(NCORES):
        out[c * BLOC:(c + 1) * BLOC] = _decode_core(res.results[c]["y"])
    return out
